# revision 11
# baseline (speedup 1.0000x reference)
"""Cross-attention Trainium2 kernel (8 NeuronCores, SPMD).

Sharding: core = 2*b + hh  (b = batch 0..3, hh = head-half 0..1).
Each core computes attention for one batch and 8 of the 16 heads, plus the
partial output projection for its head block; the host sums the two partial
projections per batch.

Per-core dataflow (all on-chip after the initial loads):
  - kT[hd, s], qT[hd, t] head-transposed projections via PE matmuls
    (host supplies tgt/src pre-transposed to [D, T] so d sits on partitions)
  - scoresT[s, t] = kT.T-slice @ qT  (two heads packed via PE row tiling)
  - p = exp(scores/8 + mask_bias)    (mask enters as the per-partition
    activation bias: s is the partition axis of scoresT)
  - attn@V with stationary [v | ones]: row 64 of the PSUM accumulator is the
    softmax denominator for free
  - normalize with a K=1 broadcast matmul + DVE multiply
  - partial out-projection, PSUM DMA'd straight to DRAM
"""

import os
import sys

import numpy as np


def _ensure_paths():
    for p in ("/opt/trn_rl_repo", "/root/.axon_site/_ro/trn_rl_repo"):
        if os.path.isdir(p) and p not in sys.path:
            sys.path.insert(0, p)


_ensure_paths()

import concourse.bass as bass  # noqa: E402
import concourse.mybir as mybir  # noqa: E402
from concourse import bacc  # noqa: E402
from concourse.bass_utils import run_bass_kernel_spmd  # noqa: E402
from concourse.tile import TileContext  # noqa: E402

B, S, T, D, H = 4, 2048, 2048, 1024, 16
HD = D // H  # 64
HL = H // 2  # heads per core: 8
HDL = HL * HD  # 512 head dims per core
PAIRS = HL // 2  # 4 head pairs (2 heads share a 128-partition tile)
KT = D // 128  # 8 contraction k-tiles for the projections
CH = 4  # t-chunks of 512
CHW = 512
STN = S // 128  # 16 source tiles
F32 = mybir.dt.float32
F32R = mybir.dt.float32r

N_CORES = 8

_PROG = None
_last_in_maps = None




def _build_program():
    nc = bacc.Bacc(None, target_bir_lowering=False, debug=False)

    tgtT = nc.dram_tensor("tgtT", [D, T], F32R, kind="ExternalInput")
    srcT = nc.dram_tensor("srcT", [D, S], F32R, kind="ExternalInput")
    wqT = nc.dram_tensor("wqT", [D, HDL], F32R, kind="ExternalInput")
    wkT = nc.dram_tensor("wkT", [D, HDL], F32R, kind="ExternalInput")
    wvT = nc.dram_tensor("wvT", [D, HDL], F32R, kind="ExternalInput")
    woS = nc.dram_tensor("woS", [HDL, D], F32R, kind="ExternalInput")
    mbias = nc.dram_tensor("mbias", [128, STN], F32, kind="ExternalInput")
    ones_in = nc.dram_tensor("ones_in", [128, 128], F32R, kind="ExternalInput")
    outp = nc.dram_tensor("outp", [T, D], F32, kind="ExternalOutput")

    Exp = mybir.ActivationFunctionType.Exp

    with nc.allow_low_precision("fp32r matmul inputs"), TileContext(nc) as tc:
        with (
            tc.tile_pool(name="const", bufs=1) as const_pool,
            tc.tile_pool(name="kv", bufs=1) as kv_pool,
            tc.tile_pool(name="pp_ps", bufs=2, space="PSUM") as pp_ps,
        ):
            # constants / weights
            WO = const_pool.tile([128, PAIRS, D], F32R)
            nc.sync.dma_start(out=WO, in_=woS[:, :].rearrange("(j p) d -> p j d", p=128))
            MB = const_pool.tile([128, STN], F32)
            nc.sync.dma_start(out=MB, in_=mbias[:, :])
            ONES = const_pool.tile([1, 64], F32R)
            nc.sync.dma_start(out=ONES, in_=ones_in[0:1, 0:64])

            # persistent K/V for the attention phase
            KTt = kv_pool.tile([128, PAIRS, S], F32R)
            VON = kv_pool.tile([128, STN, HL * (HD + 1)], F32R)
            von_heads = VON[:, :, :].rearrange("p s (h e) -> p s h e", e=HD + 1)
            nc.sync.dma_start(
                out=von_heads[:, :, :, HD],
                in_=ones_in[:, 0:128].rearrange("p (s h) -> p s h", s=STN),
            )

            # ---- source-side projections: kT and v ----
            with (
                tc.tile_pool(name="wkv", bufs=1) as wkv_pool,
                tc.tile_pool(name="s_stream", bufs=2) as s_stream,
            ):
                WK = wkv_pool.tile([128, KT, HDL], F32R)
                nc.sync.dma_start(out=WK, in_=wkT[:, :].rearrange("(k p) h -> p k h", p=128))
                WV = wkv_pool.tile([128, KT, HDL], F32R)
                nc.sync.dma_start(out=WV, in_=wvT[:, :].rearrange("(k p) h -> p k h", p=128))

                for c in range(CH):
                    SRC = s_stream.tile([128, KT, CHW], F32R)
                    nc.sync.dma_start(
                        out=SRC,
                        in_=srcT[:, :].rearrange("(k p) s -> p k s", p=128)[
                            :, :, c * CHW : (c + 1) * CHW
                        ],
                    )
                    # kT tiles for this s-chunk
                    for j in range(PAIRS):
                        k_ps = pp_ps.tile([128, CHW], F32, tag="pp")
                        for k in range(KT):
                            nc.tensor.matmul(
                                k_ps,
                                lhsT=(WK[:, k, j * 128 : (j + 1) * 128]),
                                rhs=(SRC[:, k, :]),
                                start=(k == 0),
                                stop=(k == KT - 1),
                            )
                        nc.vector.tensor_copy(KTt[:, j, c * CHW : (c + 1) * CHW], k_ps)
                    # v tiles ([s, hd] layout, head-strided with a ones column)
                    for stl in range(4):
                        st = c * 4 + stl
                        v_ps = pp_ps.tile([128, HDL], F32, tag="pp")
                        for k in range(KT):
                            nc.tensor.matmul(
                                v_ps,
                                lhsT=(SRC[:, k, stl * 128 : (stl + 1) * 128]),
                                rhs=(WV[:, k, :]),
                                start=(k == 0),
                                stop=(k == KT - 1),
                            )
                        nc.vector.tensor_copy(
                            von_heads[:, st, :, 0:HD],
                            v_ps[:, :].rearrange("p (h e) -> p h e", e=HD),
                        )

            # ---- target-side projections + attention + out-projection ----
            with (
                tc.tile_pool(name="wq", bufs=1) as wq_pool,
                tc.tile_pool(name="t_stream", bufs=2) as t_stream,
                tc.tile_pool(name="qc", bufs=2) as qc_pool,
                tc.tile_pool(name="pt", bufs=3) as pt_pool,
                tc.tile_pool(name="on", bufs=2) as on_pool,
                tc.tile_pool(name="stg", bufs=2) as stg_pool,
                tc.tile_pool(name="osb", bufs=2) as osb_pool,
                tc.tile_pool(name="rc", bufs=2) as rc_pool,
                tc.tile_pool(name="bcs", bufs=2) as bcs_pool,
                tc.tile_pool(name="sc_ps", bufs=2, space="PSUM") as sc_ps_pool,
                tc.tile_pool(name="av_ps", bufs=1, space="PSUM") as av_ps_pool,
            ):
                WQ = wq_pool.tile([128, KT, HDL], F32R)
                nc.sync.dma_start(out=WQ, in_=wqT[:, :].rearrange("(k p) h -> p k h", p=128))

                for c in range(CH):
                    TGT = t_stream.tile([128, KT, CHW], F32R)
                    nc.sync.dma_start(
                        out=TGT,
                        in_=tgtT[:, :].rearrange("(k p) t -> p k t", p=128)[
                            :, :, c * CHW : (c + 1) * CHW
                        ],
                    )
                    QTc = qc_pool.tile([128, PAIRS, CHW], F32R)
                    for j in range(PAIRS):
                        q_ps = pp_ps.tile([128, CHW], F32, tag="pp")
                        for k in range(KT):
                            nc.tensor.matmul(
                                q_ps,
                                lhsT=(WQ[:, k, j * 128 : (j + 1) * 128]),
                                rhs=(TGT[:, k, :]),
                                start=(k == 0),
                                stop=(k == KT - 1),
                            )
                        nc.vector.tensor_copy(QTc[:, j, :], q_ps)

                    OTN = on_pool.tile([128, PAIRS, CHW], F32R)
                    for j in range(PAIRS):
                        av = av_ps_pool.tile([128, 2 * CHW], F32)
                        for st in range(STN):
                            sc = sc_ps_pool.tile([128, 2 * CHW], F32, tag="sc")
                            # two heads packed via PE row tiling (K=64 each)
                            nc.tensor.matmul(
                                sc[:, 0:CHW],
                                lhsT=(KTt[0:64, j, st * 128 : (st + 1) * 128]),
                                rhs=(QTc[0:64, j, :]),
                                start=True,
                                stop=True,
                            )
                            nc.tensor.matmul(
                                sc[:, CHW : 2 * CHW],
                                lhsT=(KTt[64:128, j, st * 128 : (st + 1) * 128]),
                                rhs=(QTc[64:128, j, :]),
                                start=True,
                                stop=True,
                            )
                            PT = pt_pool.tile([128, 2 * CHW], F32R)
                            nc.scalar.activation(
                                PT, sc, Exp, bias=MB[:, st : st + 1], scale=1.0 / 8.0
                            )
                            # attn @ [v | ones]; row 64 accumulates the denominator
                            nc.tensor.matmul(
                                av[0:65, 0:CHW],
                                lhsT=(VON[:, st, j * 130 : j * 130 + 65]),
                                rhs=(PT[:, 0:CHW]),
                                start=(st == 0),
                                stop=(st == STN - 1),
                            )
                            nc.tensor.matmul(
                                av[0:65, CHW : 2 * CHW],
                                lhsT=(VON[:, st, j * 130 + 65 : j * 130 + 130]),
                                rhs=(PT[:, CHW : 2 * CHW]),
                                start=(st == 0),
                                stop=(st == STN - 1),
                            )
                        RC = rc_pool.tile([1, 2 * CHW], F32R)
                        nc.vector.reciprocal(RC, av[64:65, :])
                        BC = sc_ps_pool.tile([128, 2 * CHW], F32, tag="sc")
                        nc.tensor.matmul(
                            BC[0:64, 0:CHW],
                            lhsT=(ONES[0:1, :]),
                            rhs=(RC[0:1, 0:CHW]),
                            start=True,
                            stop=True,
                        )
                        nc.tensor.matmul(
                            BC[0:64, CHW : 2 * CHW],
                            lhsT=(ONES[0:1, :]),
                            rhs=(RC[0:1, CHW : 2 * CHW]),
                            start=True,
                            stop=True,
                        )
                        BCS = bcs_pool.tile([64, 2 * CHW], F32)
                        nc.vector.tensor_copy(BCS, BC[0:64, :])
                        nc.vector.tensor_mul(
                            OTN[0:64, j, :], av[0:64, 0:CHW], BCS[:, 0:CHW]
                        )
                        STG = stg_pool.tile([64, CHW], F32R)
                        nc.vector.tensor_mul(
                            STG, av[0:64, CHW : 2 * CHW], BCS[:, CHW : 2 * CHW]
                        )
                        nc.sync.dma_start(out=OTN[64:128, j, :], in_=STG)

                    # partial out-projection for this t-chunk
                    for ttl in range(4):
                        for dc in range(2):
                            o_ps = pp_ps.tile([128, CHW], F32, tag="pp")
                            for j in range(PAIRS):
                                nc.tensor.matmul(
                                    o_ps,
                                    lhsT=(OTN[:, j, ttl * 128 : (ttl + 1) * 128]),
                                    rhs=(WO[:, j, dc * CHW : (dc + 1) * CHW]),
                                    start=(j == 0),
                                    stop=(j == PAIRS - 1),
                                )
                            OSB = osb_pool.tile([128, CHW], F32)
                            nc.vector.tensor_copy(OSB, o_ps)
                            row0 = c * CHW + ttl * 128
                            nc.sync.dma_start(
                                out=outp[row0 : row0 + 128, dc * CHW : (dc + 1) * CHW],
                                in_=OSB,
                            )

    nc.finalize()
    return nc


def _get_program():
    global _PROG
    if _PROG is None:
        _PROG = _build_program()
    return _PROG


def kernel(src, tgt, attention_mask, Wq, Wk, Wv, Wo):
    src = np.asarray(src, dtype=np.float32)
    tgt = np.asarray(tgt, dtype=np.float32)
    mask = np.asarray(attention_mask)
    Wq = np.asarray(Wq, dtype=np.float32)
    Wk = np.asarray(Wk, dtype=np.float32)
    Wv = np.asarray(Wv, dtype=np.float32)
    Wo = np.asarray(Wo, dtype=np.float32)

    nc = _get_program()

    in_maps = []
    for core in range(N_CORES):
        b, hh = core // 2, core % 2
        rows = slice(hh * HDL, (hh + 1) * HDL)
        mb = np.where(mask[b], 0.0, -30000.0).astype(np.float32)
        in_maps.append(
            {
                "tgtT": np.ascontiguousarray(tgt[b].T),
                "srcT": np.ascontiguousarray(src[b].T),
                "wqT": np.ascontiguousarray(Wq[rows, :].T),
                "wkT": np.ascontiguousarray(Wk[rows, :].T),
                "wvT": np.ascontiguousarray(Wv[rows, :].T),
                "woS": np.ascontiguousarray(Wo[:, rows].T),
                "mbias": np.ascontiguousarray(mb.reshape(STN, 128).T),
                "ones_in": np.ones((128, 128), dtype=np.float32),
            }
        )

    global _last_in_maps
    _last_in_maps = in_maps

    res = run_bass_kernel_spmd(nc, in_maps, list(range(N_CORES)))

    out = np.empty((B, T, D), dtype=np.float32)
    for b in range(B):
        out[b] = res.results[2 * b]["outp"] + res.results[2 * b + 1]["outp"]
    return out


# revision 13
# speedup vs baseline: 1.1649x; 1.1649x over previous
"""Cross-attention Trainium2 kernel (8 NeuronCores, SPMD).

Sharding: core = 2*b + hh  (b = batch 0..3, hh = head-half 0..1).
Each core computes attention for one batch and 8 of the 16 heads, plus the
partial output projection for its head block; the host sums the two partial
projections per batch.

Per-core dataflow (all on-chip after the initial loads):
  - kT[hd, s], qT[hd, t] head-transposed projections via PE matmuls
    (host supplies tgt/src pre-transposed to [D, T] so d sits on partitions)
  - scoresT[s, t] = kT.T-slice @ qT  (two heads packed via PE row tiling)
  - p = exp(scores/8 + mask_bias)    (mask enters as the per-partition
    activation bias: s is the partition axis of scoresT)
  - attn@V with stationary [v | ones]: row 64 of the PSUM accumulator is the
    softmax denominator for free
  - normalize with a K=1 broadcast matmul + DVE multiply
  - partial out-projection, PSUM DMA'd straight to DRAM
"""

import os
import sys

import numpy as np


def _ensure_paths():
    for p in ("/opt/trn_rl_repo", "/root/.axon_site/_ro/trn_rl_repo"):
        if os.path.isdir(p) and p not in sys.path:
            sys.path.insert(0, p)


_ensure_paths()

import concourse.bass as bass  # noqa: E402
import concourse.mybir as mybir  # noqa: E402
from concourse import bacc  # noqa: E402
from concourse.bass_utils import run_bass_kernel_spmd  # noqa: E402
from concourse.tile import TileContext  # noqa: E402

B, S, T, D, H = 4, 2048, 2048, 1024, 16
HD = D // H  # 64
HL = H // 2  # heads per core: 8
HDL = HL * HD  # 512 head dims per core
PAIRS = HL // 2  # 4 head pairs (2 heads share a 128-partition tile)
KT = D // 128  # 8 contraction k-tiles for the projections
CH = 4  # t-chunks of 512
CHW = 512
STN = S // 128  # 16 source tiles
F32 = mybir.dt.float32
F32R = mybir.dt.float32r

N_CORES = 8

_PROG = None
_last_in_maps = None




def _build_program():
    nc = bacc.Bacc(None, target_bir_lowering=False, debug=False)

    tgtT = nc.dram_tensor("tgtT", [D, T], F32R, kind="ExternalInput")
    srcT = nc.dram_tensor("srcT", [D, S], F32R, kind="ExternalInput")
    wqT = nc.dram_tensor("wqT", [D, HDL], F32R, kind="ExternalInput")
    wkT = nc.dram_tensor("wkT", [D, HDL], F32R, kind="ExternalInput")
    wvT = nc.dram_tensor("wvT", [D, HDL], F32R, kind="ExternalInput")
    woS = nc.dram_tensor("woS", [HDL, D], F32R, kind="ExternalInput")
    mbias = nc.dram_tensor("mbias", [128, STN], F32, kind="ExternalInput")
    ones_in = nc.dram_tensor("ones_in", [128, 128], F32R, kind="ExternalInput")
    outp = nc.dram_tensor("outp", [T, D], F32, kind="ExternalOutput")

    Exp = mybir.ActivationFunctionType.Exp

    with nc.allow_low_precision("fp32r matmul inputs"), TileContext(nc) as tc:
        with (
            tc.tile_pool(name="const", bufs=1) as const_pool,
            tc.tile_pool(name="kv", bufs=1) as kv_pool,
            tc.tile_pool(name="acc_ps", bufs=2, space="PSUM") as acc_ps,
        ):
            # constants / weights
            WO = const_pool.tile([128, PAIRS, D], F32R)
            nc.sync.dma_start(out=WO, in_=woS[:, :].rearrange("(j p) d -> p j d", p=128))
            MB = const_pool.tile([128, STN], F32)
            nc.sync.dma_start(out=MB, in_=mbias[:, :])

            # persistent K/V for the attention phase
            KTt = kv_pool.tile([128, PAIRS, S], F32R)
            VON = kv_pool.tile([128, STN, HL * (HD + 1)], F32R)
            von_heads = VON[:, :, :].rearrange("p s (h e) -> p s h e", e=HD + 1)
            nc.sync.dma_start(
                out=von_heads[:, :, :, HD],
                in_=ones_in[:, 0:128].rearrange("p (s h) -> p s h", s=STN),
            )

            # ---- source-side projections: kT and v ----
            with (
                tc.tile_pool(name="wkv", bufs=1) as wkv_pool,
                tc.tile_pool(name="s_stream", bufs=2) as s_stream,
            ):
                WK = wkv_pool.tile([128, KT, HDL], F32R)
                nc.sync.dma_start(out=WK, in_=wkT[:, :].rearrange("(k p) h -> p k h", p=128))
                WV = wkv_pool.tile([128, KT, HDL], F32R)
                nc.sync.dma_start(out=WV, in_=wvT[:, :].rearrange("(k p) h -> p k h", p=128))

                for c in range(CH):
                    SRC = s_stream.tile([128, KT, CHW], F32R)
                    nc.sync.dma_start(
                        out=SRC,
                        in_=srcT[:, :].rearrange("(k p) s -> p k s", p=128)[
                            :, :, c * CHW : (c + 1) * CHW
                        ],
                    )
                    # kT tiles for this s-chunk
                    for j in range(PAIRS):
                        k_ps = acc_ps.tile([128, 2 * CHW], F32, tag="acc", name="k_ps")[:, 0:CHW]
                        for k in range(KT):
                            nc.tensor.matmul(
                                k_ps,
                                lhsT=(WK[:, k, j * 128 : (j + 1) * 128]),
                                rhs=(SRC[:, k, :]),
                                start=(k == 0),
                                stop=(k == KT - 1),
                            )
                        nc.vector.tensor_copy(KTt[:, j, c * CHW : (c + 1) * CHW], k_ps)
                    # v tiles ([s, hd] layout, head-strided with a ones column)
                    for stl in range(4):
                        st = c * 4 + stl
                        v_ps = acc_ps.tile([128, 2 * CHW], F32, tag="acc", name="v_ps")[:, 0:HDL]
                        for k in range(KT):
                            nc.tensor.matmul(
                                v_ps,
                                lhsT=(SRC[:, k, stl * 128 : (stl + 1) * 128]),
                                rhs=(WV[:, k, :]),
                                start=(k == 0),
                                stop=(k == KT - 1),
                            )
                        nc.vector.tensor_copy(
                            von_heads[:, st, :, 0:HD],
                            v_ps[:, :].rearrange("p (h e) -> p h e", e=HD),
                        )

            # ---- target-side projections + attention + out-projection ----
            with (
                tc.tile_pool(name="wq", bufs=1) as wq_pool,
                tc.tile_pool(name="t_stream", bufs=2) as t_stream,
                tc.tile_pool(name="qc", bufs=2) as qc_pool,
                tc.tile_pool(name="pt", bufs=3) as pt_pool,
                tc.tile_pool(name="on", bufs=2) as on_pool,
                tc.tile_pool(name="stg", bufs=2) as stg_pool,
                tc.tile_pool(name="osb", bufs=2) as osb_pool,
                tc.tile_pool(name="rc", bufs=2) as rc_pool,
                tc.tile_pool(name="bcs", bufs=2) as bcs_pool,
                tc.tile_pool(name="sc_ps", bufs=2, space="PSUM") as sc_ps_pool,
            ):
                WQ = wq_pool.tile([128, KT, HDL], F32R)
                nc.sync.dma_start(out=WQ, in_=wqT[:, :].rearrange("(k p) h -> p k h", p=128))

                for c in range(CH):
                    TGT = t_stream.tile([128, KT, CHW], F32R)
                    nc.sync.dma_start(
                        out=TGT,
                        in_=tgtT[:, :].rearrange("(k p) t -> p k t", p=128)[
                            :, :, c * CHW : (c + 1) * CHW
                        ],
                    )
                    QTc = qc_pool.tile([128, PAIRS, CHW], F32R)
                    for j in range(PAIRS):
                        q_ps = acc_ps.tile([128, 2 * CHW], F32, tag="acc", name="q_ps")[:, 0:CHW]
                        for k in range(KT):
                            nc.tensor.matmul(
                                q_ps,
                                lhsT=(WQ[:, k, j * 128 : (j + 1) * 128]),
                                rhs=(TGT[:, k, :]),
                                start=(k == 0),
                                stop=(k == KT - 1),
                            )
                        nc.vector.tensor_copy(QTc[:, j, :], q_ps)

                    OTN = on_pool.tile([128, PAIRS, CHW], F32R)
                    for j in range(PAIRS):
                        av = acc_ps.tile([128, 2 * CHW], F32, tag="acc", name="av")
                        for st in range(STN):
                            sc = sc_ps_pool.tile([128, 2 * CHW], F32, tag="sc")
                            # two heads packed via PE row tiling (K=64 each)
                            nc.tensor.matmul(
                                sc[:, 0:CHW],
                                lhsT=(KTt[0:64, j, st * 128 : (st + 1) * 128]),
                                rhs=(QTc[0:64, j, :]),
                                start=True,
                                stop=True,
                            )
                            nc.tensor.matmul(
                                sc[:, CHW : 2 * CHW],
                                lhsT=(KTt[64:128, j, st * 128 : (st + 1) * 128]),
                                rhs=(QTc[64:128, j, :]),
                                start=True,
                                stop=True,
                            )
                            PT = pt_pool.tile([128, 2 * CHW], F32R)
                            nc.scalar.activation(
                                PT, sc, Exp, bias=MB[:, st : st + 1], scale=1.0 / 8.0
                            )
                            # attn @ [v | ones]; row 64 accumulates the denominator
                            nc.tensor.matmul(
                                av[0:65, 0:CHW],
                                lhsT=(VON[:, st, j * 130 : j * 130 + 65]),
                                rhs=(PT[:, 0:CHW]),
                                start=(st == 0),
                                stop=(st == STN - 1),
                            )
                            nc.tensor.matmul(
                                av[0:65, CHW : 2 * CHW],
                                lhsT=(VON[:, st, j * 130 + 65 : j * 130 + 130]),
                                rhs=(PT[:, CHW : 2 * CHW]),
                                start=(st == 0),
                                stop=(st == STN - 1),
                            )
                        RC = rc_pool.tile([1, 2 * CHW], F32)
                        nc.vector.reciprocal(RC, av[64:65, :])
                        BCS = bcs_pool.tile([64, 2 * CHW], F32)
                        nc.gpsimd.partition_broadcast(BCS, RC[0:1, :])
                        nc.vector.tensor_mul(
                            OTN[0:64, j, :], av[0:64, 0:CHW], BCS[:, 0:CHW]
                        )
                        STG = stg_pool.tile([64, CHW], F32R)
                        nc.vector.tensor_mul(
                            STG, av[0:64, CHW : 2 * CHW], BCS[:, CHW : 2 * CHW]
                        )
                        nc.sync.dma_start(out=OTN[64:128, j, :], in_=STG)

                    # partial out-projection for this t-chunk
                    for ttl in range(4):
                        for dc in range(2):
                            o_ps = acc_ps.tile([128, 2 * CHW], F32, tag="acc", name="o_ps")[:, 0:CHW]
                            for j in range(PAIRS):
                                nc.tensor.matmul(
                                    o_ps,
                                    lhsT=(OTN[:, j, ttl * 128 : (ttl + 1) * 128]),
                                    rhs=(WO[:, j, dc * CHW : (dc + 1) * CHW]),
                                    start=(j == 0),
                                    stop=(j == PAIRS - 1),
                                )
                            OSB = osb_pool.tile([128, CHW], F32)
                            nc.vector.tensor_copy(OSB, o_ps)
                            row0 = c * CHW + ttl * 128
                            nc.sync.dma_start(
                                out=outp[row0 : row0 + 128, dc * CHW : (dc + 1) * CHW],
                                in_=OSB,
                            )

    nc.finalize()
    return nc


def _get_program():
    global _PROG
    if _PROG is None:
        _PROG = _build_program()
    return _PROG


def kernel(src, tgt, attention_mask, Wq, Wk, Wv, Wo):
    src = np.asarray(src, dtype=np.float32)
    tgt = np.asarray(tgt, dtype=np.float32)
    mask = np.asarray(attention_mask)
    Wq = np.asarray(Wq, dtype=np.float32)
    Wk = np.asarray(Wk, dtype=np.float32)
    Wv = np.asarray(Wv, dtype=np.float32)
    Wo = np.asarray(Wo, dtype=np.float32)

    nc = _get_program()

    in_maps = []
    for core in range(N_CORES):
        b, hh = core // 2, core % 2
        rows = slice(hh * HDL, (hh + 1) * HDL)
        mb = np.where(mask[b], 0.0, -30000.0).astype(np.float32)
        in_maps.append(
            {
                "tgtT": np.ascontiguousarray(tgt[b].T),
                "srcT": np.ascontiguousarray(src[b].T),
                "wqT": np.ascontiguousarray(Wq[rows, :].T),
                "wkT": np.ascontiguousarray(Wk[rows, :].T),
                "wvT": np.ascontiguousarray(Wv[rows, :].T),
                "woS": np.ascontiguousarray(Wo[:, rows].T),
                "mbias": np.ascontiguousarray(mb.reshape(STN, 128).T),
                "ones_in": np.ones((128, 128), dtype=np.float32),
            }
        )

    global _last_in_maps
    _last_in_maps = in_maps

    res = run_bass_kernel_spmd(nc, in_maps, list(range(N_CORES)))

    out = np.empty((B, T, D), dtype=np.float32)
    for b in range(B):
        out[b] = res.results[2 * b]["outp"] + res.results[2 * b + 1]["outp"]
    return out
